# revision 13
# baseline (speedup 1.0000x reference)
"""Trainium2 Bass kernel for nn_CppGraphModule_67388036874281.

Evaluates a fixed 19-node elementwise expression graph over x[2e6, 8]
(only features 0-3 used) and returns w @ nodes + bias, shape (2e6,).

Pure data parallel over 8 cores (250k samples each, padded to 128x1960,
2 chunks of 980). Host packs only the 4 used feature planes
(de-interleaved, contiguous) and folds all linear node combinations
into per-node accumulation coefficients.

Engine split per chunk:
 - ACT: the 7 transcendentals + 2 Abs + final PSUM+const output copy,
   ordered so only 2 activation-table loads happen per chunk
   (silu_and_others covers sin/silu/abs; natural_log_exp covers ln/exp).
 - DVE: 12 fused custom ops (sin range reduction with fused
   round+2-term Cody-Waite, signed-eps reciprocal via
   reciprocal_approx_fast, pow3/clip/absdiff fusions).
 - GPSIMD: 2 tensor_tensor ops offloaded (n7, n12).
 - PE: the entire 14-term weighted accumulation as fp32r diagonal
   matmuls into PSUM (1 cycle/row; node tiles are written as float32r,
   ~2.4e-4 rounding, measured end-to-end l2 ~4e-4).
"""
import sys, types

sys.path.insert(0, '/root/.axon_site')
import antenv
if not hasattr(antenv, "axon_hooks"):
    _mod = types.ModuleType("antenv.axon_hooks")
    _h = [None]
    _mod.set_axon_ntff_profile_hook = lambda h: _h.__setitem__(0, h)
    _mod.get_axon_ntff_profile_hook = lambda: _h[0]
    sys.modules["antenv.axon_hooks"] = _mod
    antenv.axon_hooks = _mod
    try:
        from trn_agent_boot.trn_boot import _ntff_profile_via_ctypes
        _mod.set_axon_ntff_profile_hook(
            _ntff_profile_via_ctypes('/opt/axon/libaxon_pjrt.so'))
    except Exception:
        pass

import numpy as np
import concourse.bacc as bacc
import concourse.mybir as mybir
from concourse.tile import TileContext
from concourse.bass_utils import run_bass_kernel_spmd

F32 = mybir.dt.float32
F32R = mybir.dt.float32r
AF = mybir.ActivationFunctionType
ALU = mybir.AluOpType

N_CORES = 8
N_TOTAL = 2_000_000
PER_CORE = N_TOTAL // N_CORES          # 250_000
FTOT = 1960                            # per-partition free dim (padded)
NCHUNK = 4
FC = FTOT // NCHUNK                    # 490
HALF = 490                             # PSUM-bank half of a chunk
EPS = 1e-10
K14 = float(1.0 / (2.5 + EPS))
SHIFT = -0.3 / 0.7                     # s17 phase pre-shift (phi/omega)

TWO_PI = 2.0 * np.pi
P1 = 512.0 * TWO_PI
MAGIC = 12582912.0                     # 1.5 * 2**23: round-to-nearest trick

# s5 = sin(1.3*x0 + 0.2): k = round(x0*C0 + C1); r5 = x0 - k*(2pi/1.3)
K5_SCALE = float(1.3 / TWO_PI)
K5_BIAS = float(0.2 / TWO_PI)
C5 = float(np.float32(TWO_PI / 1.3))


def _trunc14(v):
    f = np.float32(v)
    u = f.view(np.uint32) & np.uint32(0xFFFFFC00)
    return float(u.view(np.float32))


def _split2(v):
    c1 = _trunc14(v)
    c2 = float(np.float32(v - c1))
    return c1, c2


CA = P1 / 0.7
CA1, CA2 = _split2(CA)
FA = float(0.7 / P1)
CB = TWO_PI / 0.7
CB1, CB2 = _split2(CB)
FB = float(0.7 / TWO_PI)

# PE accumulation slots, in issue order (early-available first)
PE_SLOTS = ["x0", "x1", "x2", "x3", "n10", "s5", "L", "n9", "n12", "n7",
            "n13", "n15", "n16s", "s17"]

_CACHED_NC = None
_OPS_REGISTERED = {}


def _make_dve_op(name, spec):
    from concourse.dve_ops import DveOp, OPS, get_dve_sub_opcode, has_src1
    from concourse.dve_uop import DveOpSpec
    from concourse.dve_spec import lower
    if name in _OPS_REGISTERED:
        return _OPS_REGISTERED[name]
    for o in OPS:
        if o.name == name:
            _OPS_REGISTERED[name] = o
            return o
    import concourse.dve_ops as dve_ops_mod
    op = DveOp(name, spec, subdim=False, uops_sha={"v3": "?", "v4": "?"})
    OPS.append(op)
    dve_ops_mod._SUB_OPCODE_FOR_NAME[name] = (
        dve_ops_mod._CUSTOM_DVE_ROW_BASE + len(OPS) - 1)
    dve_ops_mod.CUSTOM_DVE_SPECS[name] = spec
    for ver in ("v3", "v4"):
        result = DveOpSpec(name=name, opcode=get_dve_sub_opcode(name),
                           uops=lower(spec, ver=ver), rd1_en=has_src1(spec))
        op.uops_sha[ver] = result.sha(ver)
    _OPS_REGISTERED[name] = op
    return op


def _register_ops():
    from concourse.dve_spec import (Spec, Src0, Src1, C0, C1, C2, C3, Zero,
                                    maxx, minn, select, sq)
    from concourse.dve_ops import _spill_c3_to_src1
    ops = {}
    # SINRED5: k = round(Src0*C0 + C1) via magic (C3->Src1); r = Src0 - k*C2
    _k5 = (Src0 * C0 + C1 + C3) - C3
    ops["SINRED5"] = _make_dve_op(
        "SINRED5_ANT", Spec(body=_spill_c3_to_src1(Src0 - _k5 * C2)))
    # CW2: k = round(x*C0) via magic (C3->Src1); out = (x - k*C1) - k*C2
    _k = ((Src0 * C0 + C3) - C3)
    ops["CW2"] = _make_dve_op(
        "CW2_ANT", Spec(body=_spill_c3_to_src1((Src0 - _k * C1) - _k * C2)))
    # SELEPS3: x>0 ? x+C0 : (x<0 ? x-C0 : C1)  (signed eps denominator)
    ops["SELEPS3"] = _make_dve_op(
        "SELEPS3_ANT",
        Spec(body=select(Src0 > Zero, Src0 + C0,
                         select(Src0 < Zero, Src0 - C0, C1))))
    # LCOMB: Src0*C0 + Src1^2*C1   (L = 1.1*K14*s5 + K14*x1^2)
    ops["LCOMB"] = _make_dve_op(
        "LCOMB_ANT", Spec(body=Src0 * C0 + sq(Src1) * C1))
    # POW3: Src0^3
    ops["POW3"] = _make_dve_op("POW3_ANT", Spec(body=sq(Src0) * Src0))
    # N16S: Src0*C0 + Src1 + C1   (n16s = 0.5*t16 + m + SHIFT)
    ops["N16S"] = _make_dve_op(
        "N16S_ANT", Spec(body=Src0 * C0 + Src1 + C1))
    # MULCLIP: clip(Src0*Src1, C0, C1)
    ops["MULCLIP"] = _make_dve_op(
        "MULCLIP_ANT", Spec(body=minn(maxx(Src0 * Src1, C0), C1)))
    # ABSDIFF: |Src0 - Src1|
    ops["ABSDIFF"] = _make_dve_op(
        "ABSDIFF_ANT", Spec(body=maxx(Src0 - Src1, Src1 - Src0)))
    return ops


def fold_coefficients(w, b):
    w = np.asarray(w, np.float64)
    b = float(b)
    c11 = w[11] + K14 * w[14] + w[18]
    c_s5 = 1.1 * (w[5] + c11)
    c_n6 = w[6] + c11
    c = {
        "x0": w[0], "x1": w[1], "x2": w[2], "x3": w[3],
        "n10": w[10],
        "s5": c_s5 - 1.1 * c_n6,
        "L": c_n6 / K14,
        "n9": w[9],
        "n12": -(w[8] - w[12]),
        "n7": (w[7] + w[12]) + (w[8] - w[12]),
        "n13": w[13],
        "n15": w[15],
        "n16s": w[16],
        "s17": 0.9 * (w[17] + w[18]),
    }
    const = b + 2.5 * w[4] - w[16] * SHIFT
    return c, const


CCOL = {"magic": 0, "const": 1, "b_s5": 2, "b_eps": 3}


DBG_TILES = ["s5f", "s5", "n9f", "n9", "e7", "n7", "n12", "n10", "n13",
             "sd", "rec", "n15", "L", "au", "m", "t16", "n16s", "rA", "r2",
             "s17f", "s17", "yout"]


def build_nc(debug_dump=False):
    ops = _register_ops()
    nc = bacc.Bacc("TRN2", target_bir_lowering=False, debug=False,
                   num_devices=N_CORES)
    x0t = nc.dram_tensor("x0", [128, NCHUNK, FC], F32,
                         kind="ExternalInput").ap()
    x = nc.dram_tensor("x", [128, NCHUNK, 3, FC], F32R,
                       kind="ExternalInput").ap()
    coefs = nc.dram_tensor("coefs", [128, 8], F32, kind="ExternalInput").ap()
    iden = nc.dram_tensor("iden", [128, len(PE_SLOTS) * 128], F32R,
                          kind="ExternalInput").ap()
    y = nc.dram_tensor("y", [128, NCHUNK, FC], F32,
                       kind="ExternalOutput").ap()
    dbg = None
    if debug_dump:
        dbg = nc.dram_tensor("dbg", [128, len(DBG_TILES), FC], F32,
                             kind="ExternalOutput").ap()

    with TileContext(nc) as tc:
        with tc.tile_pool(name="consts", bufs=1) as cpool, \
             tc.tile_pool(name="xin", bufs=2) as xpool, \
             tc.tile_pool(name="wk2", bufs=2) as wp2, \
             tc.tile_pool(name="wk1", bufs=1) as wp1, \
             tc.tile_pool(name="yo", bufs=2) as ypool, \
             tc.tile_pool(name="psum", bufs=2, space="PSUM") as ppool:

            ct = cpool.tile([128, 8], F32, name="coefs")
            nc.sync.dma_start(out=ct[:], in_=coefs[:, :])
            it = cpool.tile([128, len(PE_SLOTS) * 128], F32R, name="iden")
            iden_loaded = [False]

            magic = ct[:, CCOL["magic"]:CCOL["magic"] + 1]
            const = ct[:, CCOL["const"]:CCOL["const"] + 1]
            b_s5 = ct[:, CCOL["b_s5"]:CCOL["b_s5"] + 1]
            b_eps = ct[:, CCOL["b_eps"]:CCOL["b_eps"] + 1]

            for cix in range(NCHUNK):
                x0r = xpool.tile([128, FC], F32, tag="x0r", name="x0r")
                nc.sync.dma_start(out=x0r[:], in_=x0t[:, cix])
                xr = xpool.tile([128, 3, FC], F32R, tag="xr", name="xr")
                nc.sync.dma_start(out=xr[:], in_=x[:, cix])
                if not iden_loaded[0]:
                    nc.sync.dma_start(out=it[:], in_=iden[:, :])
                    iden_loaded[0] = True
                xcol_r = [None, xr[:, 0, :], xr[:, 1, :], xr[:, 2, :]]
                xcol = [x0r[:]] + [v.bitcast(F32) for v in xcol_r[1:]]

                ype = ppool.tile([128, FC], F32, tag="ype", name="ype")
                slot_done = {}

                def mm(name, rhs):
                    j = PE_SLOTS.index(name)
                    nmm = (FC + HALF - 1) // HALF
                    for h in range(nmm):
                        lo, hi = h * HALF, min(FC, (h + 1) * HALF)
                        nc.tensor.matmul(
                            ype[:, lo:hi],
                            it[:, j * 128:(j + 1) * 128],
                            rhs[:, lo:hi],
                            start=(j == 0), stop=(j == len(PE_SLOTS) - 1))
                    slot_done[name] = True

                def t2(tag, dt=F32R):
                    return wp2.tile([128, FC], dt, tag=tag, name=tag)

                def t1(tag, dt=F32):
                    return wp1.tile([128, FC], dt, tag=tag, name=tag)

                # x slots go to PE immediately (x0 via GPSIMD f32r copy)
                xq0 = xpool.tile([128, FC], F32R, tag="xq0", name="xq0")
                nc.gpsimd.dma_start(out=xq0[:], in_=x0r[:])
                mm("x0", xq0)
                mm("x1", xcol_r[1])
                mm("x2", xcol_r[2])
                mm("x3", xcol_r[3])

                # --- DVE early ---
                r5 = t1("r5")
                nc.vector._custom_dve(ops["SINRED5"], out=r5[:], in0=xcol[0],
                                      in1=magic, s0=K5_SCALE, s1=K5_BIAS,
                                      imm2=C5)
                n10 = t2("n10")
                nc.vector._custom_dve(ops["POW3"], out=n10[:], in0=xcol[1])
                mm("n10", n10)

                # --- ACT set-A head (abs fillers + s5) ---
                a0 = t1("a0")
                nc.scalar.activation(a0[:], xcol[0], AF.Abs)
                a2 = t1("a2")
                nc.scalar.activation(a2[:], xcol[2], AF.Abs)
                s5f = t1("s5f")
                nc.scalar.activation(s5f[:], r5[:], AF.Sin, bias=b_s5,
                                     scale=1.3)
                s5 = t2("s5")
                nc.gpsimd.dma_start(out=s5[:], in_=s5f[:])
                mm("s5", s5)

                L = t2("L")
                nc.vector._custom_dve(ops["LCOMB"], out=L[:], in0=s5f[:],
                                      in1=xcol[1], s0=1.1 * K14, s1=K14)
                mm("L", L)

                # --- ACT set-B block (ln/exp) ---
                l7 = t1("l7")
                nc.scalar.activation(l7[:], a2[:], AF.Ln, bias=b_eps)
                n9f = t1("n9f")
                nc.scalar.activation(n9f[:], a0[:], AF.Ln, bias=b_eps)
                n9 = t2("n9")
                nc.gpsimd.dma_start(out=n9[:], in_=n9f[:])
                mm("n9", n9)
                e7 = t1("e7")
                nc.scalar.activation(e7[:], l7[:], AF.Exp, scale=0.7)
                n8 = t1("n8")
                nc.scalar.activation(n8[:], xcol[3], AF.Exp, scale=0.5)

                # --- GPSIMD offload: n7 = x2*e7, n12 = n7 - n8 (both F32R) ---
                n7 = t2("n7")
                nc.gpsimd.tensor_tensor(n7[:], xcol[2], e7[:], ALU.mult)
                mm("n7", n7)
                n12 = t2("n12")
                nc.gpsimd.tensor_tensor(n12[:], n7[:].bitcast(F32),
                                        n8[:], ALU.subtract)
                mm("n12", n12)

                # --- DVE division chain ---
                n13 = t2("n13")
                nc.vector.tensor_mul(n13[:], n9f[:],
                                     n10[:].bitcast(F32))
                mm("n13", n13)
                sd = t1("sd")
                nc.vector._custom_dve(ops["SELEPS3"], out=sd[:],
                                      in0=n13[:].bitcast(F32), s0=EPS, s1=1e30)
                rec = t1("rec")
                nc.vector.reciprocal_approx_fast(out=rec[:], in_=sd[:])
                n15 = t2("n15")
                nc.vector._custom_dve(ops["MULCLIP"], out=n15[:],
                                      in0=n12[:].bitcast(F32),
                                      in1=rec[:], s0=-1e6, s1=1e6)
                mm("n15", n15)

                # --- aggregation ---
                au = t1("au")
                nc.vector._custom_dve(ops["ABSDIFF"], out=au[:],
                                      in0=L[:].bitcast(F32),
                                      in1=n15[:].bitcast(F32))
                m = t1("m")
                nc.vector.tensor_tensor(m[:], L[:].bitcast(F32),
                                        n15[:].bitcast(F32), ALU.max)
                t16 = t1("t16")
                nc.scalar.activation(t16[:], au[:], AF.Silu, scale=-2.0)
                n16s = t2("n16s")
                nc.vector._custom_dve(ops["N16S"], out=n16s[:], in0=t16[:],
                                      in1=m[:], s0=0.5, s1=SHIFT)
                mm("n16s", n16s)

                # --- s17 range reduction + sin ---
                rA = t1("rA")
                nc.vector._custom_dve(ops["CW2"], out=rA[:],
                                      in0=n16s[:].bitcast(F32), in1=magic,
                                      s0=FA, s1=CA1, imm2=CA2)
                r2 = t1("r2")
                nc.vector._custom_dve(ops["CW2"], out=r2[:], in0=rA[:],
                                      in1=magic, s0=FB, s1=CB1, imm2=CB2)
                s17f = t1("s17f")
                nc.scalar.activation(s17f[:], r2[:], AF.Sin, scale=0.7)
                s17 = t2("s17")
                nc.gpsimd.dma_start(out=s17[:], in_=s17f[:])
                mm("s17", s17)

                assert len(slot_done) == len(PE_SLOTS)

                # --- output: y = ype + const (ACT, PSUM->SBUF) + DMA out ---
                yout = ypool.tile([128, FC], F32, tag="yout", name="yout")
                nc.scalar.activation(yout[:], ype[:], AF.Identity, bias=const)
                nc.sync.dma_start(out=y[:, cix], in_=yout[:])
                if debug_dump and cix == 0:
                    loc = dict(locals())
                    for di, dn in enumerate(DBG_TILES):
                        t = loc[dn]
                        nc.sync.dma_start(out=dbg[:, di],
                                          in_=t[:].bitcast(F32))
    nc.compile()
    return nc


def _prepare_inputs(x, output_weights, output_bias):
    c, const = fold_coefficients(output_weights, output_bias)
    coefrow = np.zeros(8, np.float32)
    coefrow[CCOL["magic"]] = MAGIC
    coefrow[CCOL["const"]] = const
    coefrow[CCOL["b_s5"]] = 0.2
    coefrow[CCOL["b_eps"]] = EPS
    coefs = np.tile(coefrow, (128, 1))

    iden = np.zeros((128, len(PE_SLOTS) * 128), np.float32)
    for j, k in enumerate(PE_SLOTS):
        np.fill_diagonal(iden[:, j * 128:(j + 1) * 128], np.float32(c[k]))

    in_maps = []
    for core in range(N_CORES):
        xc = np.zeros((128 * FTOT, 4), np.float32)
        xc[:PER_CORE] = x[core * PER_CORE:(core + 1) * PER_CORE, :4]
        # [128*FTOT, 4] -> [128, NCHUNK, FC, 4] -> [128, NCHUNK, 4, FC]
        xp = xc.reshape(128, NCHUNK, FC, 4).transpose(0, 1, 3, 2)
        in_maps.append({
            "x0": np.ascontiguousarray(xp[:, :, 0]),
            "x": np.ascontiguousarray(xp[:, :, 1:]),
            "coefs": coefs,
            "iden": iden,
        })
    return in_maps


def kernel(x, output_weights, output_bias):
    global _CACHED_NC
    if _CACHED_NC is None:
        _CACHED_NC = build_nc()
    nc = _CACHED_NC
    in_maps = _prepare_inputs(np.asarray(x, np.float32),
                              output_weights, output_bias)
    res = run_bass_kernel_spmd(nc, in_maps, core_ids=list(range(N_CORES)))
    outs = []
    for core in range(N_CORES):
        yc = res.results[core]["y"].reshape(-1)[:PER_CORE]
        outs.append(yc)
    return np.concatenate(outs).astype(np.float64)


# revision 14
# speedup vs baseline: 1.0813x; 1.0813x over previous
"""Trainium2 Bass kernel for nn_CppGraphModule_67388036874281.

Evaluates a fixed 19-node elementwise expression graph over x[2e6, 8]
(only features 0-3 used) and returns w @ nodes + bias, shape (2e6,).

Pure data parallel over 8 cores (250k samples each, padded to 128x1960,
2 chunks of 980). Host packs only the 4 used feature planes
(de-interleaved, contiguous) and folds all linear node combinations
into per-node accumulation coefficients.

Engine split per chunk:
 - ACT: the 7 transcendentals + 2 Abs + final PSUM+const output copy,
   ordered so only 2 activation-table loads happen per chunk
   (silu_and_others covers sin/silu/abs; natural_log_exp covers ln/exp).
 - DVE: 12 fused custom ops (sin range reduction with fused
   round+2-term Cody-Waite, signed-eps reciprocal via
   reciprocal_approx_fast, pow3/clip/absdiff fusions).
 - GPSIMD: 2 tensor_tensor ops offloaded (n7, n12).
 - PE: the entire 14-term weighted accumulation as fp32r diagonal
   matmuls into PSUM (1 cycle/row; node tiles are written as float32r,
   ~2.4e-4 rounding, measured end-to-end l2 ~4e-4).
"""
import sys, types

sys.path.insert(0, '/root/.axon_site')
import antenv
if not hasattr(antenv, "axon_hooks"):
    _mod = types.ModuleType("antenv.axon_hooks")
    _h = [None]
    _mod.set_axon_ntff_profile_hook = lambda h: _h.__setitem__(0, h)
    _mod.get_axon_ntff_profile_hook = lambda: _h[0]
    sys.modules["antenv.axon_hooks"] = _mod
    antenv.axon_hooks = _mod
    try:
        from trn_agent_boot.trn_boot import _ntff_profile_via_ctypes
        _mod.set_axon_ntff_profile_hook(
            _ntff_profile_via_ctypes('/opt/axon/libaxon_pjrt.so'))
    except Exception:
        pass

import numpy as np
import concourse.bacc as bacc
import concourse.mybir as mybir
from concourse.tile import TileContext
from concourse.bass_utils import run_bass_kernel_spmd

F32 = mybir.dt.float32
F32R = mybir.dt.float32r
AF = mybir.ActivationFunctionType
ALU = mybir.AluOpType

N_CORES = 8
N_TOTAL = 2_000_000
PER_CORE = N_TOTAL // N_CORES          # 250_000
FTOT = 1960                            # per-partition free dim (padded)
NCHUNK = 2
FC = FTOT // NCHUNK                    # 980
HALF = 490                             # PSUM-bank half of a chunk
EPS = 1e-10
K14 = float(1.0 / (2.5 + EPS))
SHIFT = -0.3 / 0.7                     # s17 phase pre-shift (phi/omega)

TWO_PI = 2.0 * np.pi
P1 = 512.0 * TWO_PI
MAGIC = 12582912.0                     # 1.5 * 2**23: round-to-nearest trick

# s5 = sin(1.3*x0 + 0.2): k = round(x0*C0 + C1); r5 = x0 - k*(2pi/1.3)
K5_SCALE = float(1.3 / TWO_PI)
K5_BIAS = float(0.2 / TWO_PI)
C5 = float(np.float32(TWO_PI / 1.3))


def _trunc14(v):
    f = np.float32(v)
    u = f.view(np.uint32) & np.uint32(0xFFFFFC00)
    return float(u.view(np.float32))


def _split2(v):
    c1 = _trunc14(v)
    c2 = float(np.float32(v - c1))
    return c1, c2


CA = P1 / 0.7
CA1, CA2 = _split2(CA)
FA = float(0.7 / P1)
CB = TWO_PI / 0.7
CB1, CB2 = _split2(CB)
FB = float(0.7 / TWO_PI)

# PE accumulation slots, in issue order (early-available first)
PE_SLOTS = ["x0", "x1", "x2", "x3", "n10", "s5", "L", "n9", "n12", "n7",
            "n13", "n15", "n16s", "s17"]

_CACHED_NC = None
_OPS_REGISTERED = {}


def _make_dve_op(name, spec):
    from concourse.dve_ops import DveOp, OPS, get_dve_sub_opcode, has_src1
    from concourse.dve_uop import DveOpSpec
    from concourse.dve_spec import lower
    if name in _OPS_REGISTERED:
        return _OPS_REGISTERED[name]
    for o in OPS:
        if o.name == name:
            _OPS_REGISTERED[name] = o
            return o
    import concourse.dve_ops as dve_ops_mod
    op = DveOp(name, spec, subdim=False, uops_sha={"v3": "?", "v4": "?"})
    OPS.append(op)
    dve_ops_mod._SUB_OPCODE_FOR_NAME[name] = (
        dve_ops_mod._CUSTOM_DVE_ROW_BASE + len(OPS) - 1)
    dve_ops_mod.CUSTOM_DVE_SPECS[name] = spec
    for ver in ("v3", "v4"):
        result = DveOpSpec(name=name, opcode=get_dve_sub_opcode(name),
                           uops=lower(spec, ver=ver), rd1_en=has_src1(spec))
        op.uops_sha[ver] = result.sha(ver)
    _OPS_REGISTERED[name] = op
    return op


def _register_ops():
    from concourse.dve_spec import (Spec, Src0, Src1, C0, C1, C2, C3, Zero,
                                    maxx, minn, select, sq)
    from concourse.dve_ops import _spill_c3_to_src1
    ops = {}
    # SINRED5: k = round(Src0*C0 + C1) via magic (C3->Src1); r = Src0 - k*C2
    _k5 = (Src0 * C0 + C1 + C3) - C3
    ops["SINRED5"] = _make_dve_op(
        "SINRED5_ANT", Spec(body=_spill_c3_to_src1(Src0 - _k5 * C2)))
    # CW2: k = round(x*C0) via magic (C3->Src1); out = (x - k*C1) - k*C2
    _k = ((Src0 * C0 + C3) - C3)
    ops["CW2"] = _make_dve_op(
        "CW2_ANT", Spec(body=_spill_c3_to_src1((Src0 - _k * C1) - _k * C2)))
    # SELEPS3: x>0 ? x+C0 : (x<0 ? x-C0 : C1)  (signed eps denominator)
    ops["SELEPS3"] = _make_dve_op(
        "SELEPS3_ANT",
        Spec(body=select(Src0 > Zero, Src0 + C0,
                         select(Src0 < Zero, Src0 - C0, C1))))
    # LCOMB: Src0*C0 + Src1^2*C1   (L = 1.1*K14*s5 + K14*x1^2)
    ops["LCOMB"] = _make_dve_op(
        "LCOMB_ANT", Spec(body=Src0 * C0 + sq(Src1) * C1))
    # POW3: Src0^3
    ops["POW3"] = _make_dve_op("POW3_ANT", Spec(body=sq(Src0) * Src0))
    # N16S: Src0*C0 + Src1 + C1   (n16s = 0.5*t16 + m + SHIFT)
    ops["N16S"] = _make_dve_op(
        "N16S_ANT", Spec(body=Src0 * C0 + Src1 + C1))
    # MULCLIP: clip(Src0*Src1, C0, C1)
    ops["MULCLIP"] = _make_dve_op(
        "MULCLIP_ANT", Spec(body=minn(maxx(Src0 * Src1, C0), C1)))
    # ABSDIFF: |Src0 - Src1|
    ops["ABSDIFF"] = _make_dve_op(
        "ABSDIFF_ANT", Spec(body=maxx(Src0 - Src1, Src1 - Src0)))
    return ops


def fold_coefficients(w, b):
    w = np.asarray(w, np.float64)
    b = float(b)
    c11 = w[11] + K14 * w[14] + w[18]
    c_s5 = 1.1 * (w[5] + c11)
    c_n6 = w[6] + c11
    c = {
        "x0": w[0], "x1": w[1], "x2": w[2], "x3": w[3],
        "n10": w[10],
        "s5": c_s5 - 1.1 * c_n6,
        "L": c_n6 / K14,
        "n9": w[9],
        "n12": -(w[8] - w[12]),
        "n7": (w[7] + w[12]) + (w[8] - w[12]),
        "n13": w[13],
        "n15": w[15],
        "n16s": w[16],
        "s17": 0.9 * (w[17] + w[18]),
    }
    const = b + 2.5 * w[4] - w[16] * SHIFT
    return c, const


CCOL = {"magic": 0, "const": 1, "b_s5": 2, "b_eps": 3}


DBG_TILES = ["s5f", "s5", "n9f", "n9", "e7", "n7", "n12", "n10", "n13",
             "sd", "rec", "n15", "L", "au", "m", "t16", "n16s", "rA", "r2",
             "s17f", "s17", "yout"]


def build_nc(debug_dump=False):
    ops = _register_ops()
    nc = bacc.Bacc("TRN2", target_bir_lowering=False, debug=False,
                   num_devices=N_CORES)
    x0t = nc.dram_tensor("x0", [128, NCHUNK, FC], F32,
                         kind="ExternalInput").ap()
    x = nc.dram_tensor("x", [128, NCHUNK, 3, FC], F32R,
                       kind="ExternalInput").ap()
    coefs = nc.dram_tensor("coefs", [128, 8], F32, kind="ExternalInput").ap()
    iden = nc.dram_tensor("iden", [128, len(PE_SLOTS) * 128], F32R,
                          kind="ExternalInput").ap()
    y = nc.dram_tensor("y", [128, NCHUNK, FC], F32,
                       kind="ExternalOutput").ap()
    dbg = None
    if debug_dump:
        dbg = nc.dram_tensor("dbg", [128, len(DBG_TILES), FC], F32,
                             kind="ExternalOutput").ap()

    with TileContext(nc) as tc:
        with tc.tile_pool(name="consts", bufs=1) as cpool, \
             tc.tile_pool(name="xin", bufs=2) as xpool, \
             tc.tile_pool(name="wk2", bufs=2) as wp2, \
             tc.tile_pool(name="wk1", bufs=1) as wp1, \
             tc.tile_pool(name="yo", bufs=2) as ypool, \
             tc.tile_pool(name="psum", bufs=2, space="PSUM") as ppool:

            ct = cpool.tile([128, 8], F32, name="coefs")
            nc.sync.dma_start(out=ct[:], in_=coefs[:, :])
            it = cpool.tile([128, len(PE_SLOTS) * 128], F32R, name="iden")
            iden_loaded = [False]

            magic = ct[:, CCOL["magic"]:CCOL["magic"] + 1]
            const = ct[:, CCOL["const"]:CCOL["const"] + 1]
            b_s5 = ct[:, CCOL["b_s5"]:CCOL["b_s5"] + 1]
            b_eps = ct[:, CCOL["b_eps"]:CCOL["b_eps"] + 1]

            for cix in range(NCHUNK):
                x0r = xpool.tile([128, FC], F32, tag="x0r", name="x0r")
                nc.sync.dma_start(out=x0r[:], in_=x0t[:, cix])
                xr = xpool.tile([128, 3, FC], F32R, tag="xr", name="xr")
                nc.sync.dma_start(out=xr[:], in_=x[:, cix])
                if not iden_loaded[0]:
                    nc.sync.dma_start(out=it[:], in_=iden[:, :])
                    iden_loaded[0] = True
                xcol_r = [None, xr[:, 0, :], xr[:, 1, :], xr[:, 2, :]]
                xcol = [x0r[:]] + [v.bitcast(F32) for v in xcol_r[1:]]

                ype = ppool.tile([128, FC], F32, tag="ype", name="ype")
                slot_done = {}

                def mm(name, rhs):
                    j = PE_SLOTS.index(name)
                    nmm = (FC + HALF - 1) // HALF
                    for h in range(nmm):
                        lo, hi = h * HALF, min(FC, (h + 1) * HALF)
                        nc.tensor.matmul(
                            ype[:, lo:hi],
                            it[:, j * 128:(j + 1) * 128],
                            rhs[:, lo:hi],
                            start=(j == 0), stop=(j == len(PE_SLOTS) - 1))
                    slot_done[name] = True

                def t2(tag, dt=F32R):
                    return wp2.tile([128, FC], dt, tag=tag, name=tag)

                def t1(tag, dt=F32):
                    return wp1.tile([128, FC], dt, tag=tag, name=tag)

                # x slots go to PE immediately (x0 via GPSIMD f32r copy)
                xq0 = xpool.tile([128, FC], F32R, tag="xq0", name="xq0")
                nc.gpsimd.dma_start(out=xq0[:], in_=x0r[:])
                mm("x0", xq0)
                mm("x1", xcol_r[1])
                mm("x2", xcol_r[2])
                mm("x3", xcol_r[3])

                # --- DVE early ---
                r5 = t1("r5")
                nc.vector._custom_dve(ops["SINRED5"], out=r5[:], in0=xcol[0],
                                      in1=magic, s0=K5_SCALE, s1=K5_BIAS,
                                      imm2=C5)
                n10 = t2("n10")
                nc.vector._custom_dve(ops["POW3"], out=n10[:], in0=xcol[1])
                mm("n10", n10)

                # --- ACT set-A head (abs fillers + s5) ---
                a0 = t1("a0")
                nc.scalar.activation(a0[:], xcol[0], AF.Abs)
                a2 = t1("a2")
                nc.scalar.activation(a2[:], xcol[2], AF.Abs)
                s5f = t1("s5f")
                nc.scalar.activation(s5f[:], r5[:], AF.Sin, bias=b_s5,
                                     scale=1.3)
                s5 = t2("s5")
                nc.gpsimd.dma_start(out=s5[:], in_=s5f[:])
                mm("s5", s5)

                L = t2("L")
                nc.vector._custom_dve(ops["LCOMB"], out=L[:], in0=s5f[:],
                                      in1=xcol[1], s0=1.1 * K14, s1=K14)
                mm("L", L)

                # --- ACT set-B block (ln/exp) ---
                l7 = t1("l7")
                nc.scalar.activation(l7[:], a2[:], AF.Ln, bias=b_eps)
                n9f = t1("n9f")
                nc.scalar.activation(n9f[:], a0[:], AF.Ln, bias=b_eps)
                n9 = t2("n9")
                nc.gpsimd.dma_start(out=n9[:], in_=n9f[:])
                mm("n9", n9)
                e7 = t1("e7")
                nc.scalar.activation(e7[:], l7[:], AF.Exp, scale=0.7)
                n8 = t1("n8")
                nc.scalar.activation(n8[:], xcol[3], AF.Exp, scale=0.5)

                # --- GPSIMD offload: n7 = x2*e7, n12 = n7 - n8 (both F32R) ---
                n7 = t2("n7")
                nc.gpsimd.tensor_tensor(n7[:], xcol[2], e7[:], ALU.mult)
                mm("n7", n7)
                n12 = t2("n12")
                nc.gpsimd.tensor_tensor(n12[:], n7[:].bitcast(F32),
                                        n8[:], ALU.subtract)
                mm("n12", n12)

                # --- DVE division chain ---
                n13 = t2("n13")
                nc.vector.tensor_mul(n13[:], n9f[:],
                                     n10[:].bitcast(F32))
                mm("n13", n13)
                sd = t1("sd")
                nc.vector._custom_dve(ops["SELEPS3"], out=sd[:],
                                      in0=n13[:].bitcast(F32), s0=EPS, s1=1e30)
                rec = t1("rec")
                nc.vector.reciprocal_approx_fast(out=rec[:], in_=sd[:])
                n15 = t2("n15")
                nc.vector._custom_dve(ops["MULCLIP"], out=n15[:],
                                      in0=n12[:].bitcast(F32),
                                      in1=rec[:], s0=-1e6, s1=1e6)
                mm("n15", n15)

                # --- aggregation ---
                au = t1("au")
                nc.vector._custom_dve(ops["ABSDIFF"], out=au[:],
                                      in0=L[:].bitcast(F32),
                                      in1=n15[:].bitcast(F32))
                m = t1("m")
                nc.vector.tensor_tensor(m[:], L[:].bitcast(F32),
                                        n15[:].bitcast(F32), ALU.max)
                t16 = t1("t16")
                nc.scalar.activation(t16[:], au[:], AF.Silu, scale=-2.0)
                n16s = t2("n16s")
                nc.vector._custom_dve(ops["N16S"], out=n16s[:], in0=t16[:],
                                      in1=m[:], s0=0.5, s1=SHIFT)
                mm("n16s", n16s)

                # --- s17 range reduction + sin ---
                rA = t1("rA")
                nc.vector._custom_dve(ops["CW2"], out=rA[:],
                                      in0=n16s[:].bitcast(F32), in1=magic,
                                      s0=FA, s1=CA1, imm2=CA2)
                r2 = t1("r2")
                nc.vector._custom_dve(ops["CW2"], out=r2[:], in0=rA[:],
                                      in1=magic, s0=FB, s1=CB1, imm2=CB2)
                s17f = t1("s17f")
                nc.scalar.activation(s17f[:], r2[:], AF.Sin, scale=0.7)
                s17 = t2("s17")
                nc.gpsimd.dma_start(out=s17[:], in_=s17f[:])
                mm("s17", s17)

                assert len(slot_done) == len(PE_SLOTS)

                # --- output: y = ype + const (ACT, PSUM->SBUF) + DMA out ---
                yout = ypool.tile([128, FC], F32, tag="yout", name="yout")
                nc.scalar.activation(yout[:], ype[:], AF.Identity, bias=const)
                nc.sync.dma_start(out=y[:, cix], in_=yout[:])
                if debug_dump and cix == 0:
                    loc = dict(locals())
                    for di, dn in enumerate(DBG_TILES):
                        t = loc[dn]
                        nc.sync.dma_start(out=dbg[:, di],
                                          in_=t[:].bitcast(F32))
    nc.compile()
    return nc


def _prepare_inputs(x, output_weights, output_bias):
    c, const = fold_coefficients(output_weights, output_bias)
    coefrow = np.zeros(8, np.float32)
    coefrow[CCOL["magic"]] = MAGIC
    coefrow[CCOL["const"]] = const
    coefrow[CCOL["b_s5"]] = 0.2
    coefrow[CCOL["b_eps"]] = EPS
    coefs = np.tile(coefrow, (128, 1))

    iden = np.zeros((128, len(PE_SLOTS) * 128), np.float32)
    for j, k in enumerate(PE_SLOTS):
        np.fill_diagonal(iden[:, j * 128:(j + 1) * 128], np.float32(c[k]))

    in_maps = []
    for core in range(N_CORES):
        xc = np.zeros((128 * FTOT, 4), np.float32)
        xc[:PER_CORE] = x[core * PER_CORE:(core + 1) * PER_CORE, :4]
        # [128*FTOT, 4] -> [128, NCHUNK, FC, 4] -> [128, NCHUNK, 4, FC]
        xp = xc.reshape(128, NCHUNK, FC, 4).transpose(0, 1, 3, 2)
        in_maps.append({
            "x0": np.ascontiguousarray(xp[:, :, 0]),
            "x": np.ascontiguousarray(xp[:, :, 1:]),
            "coefs": coefs,
            "iden": iden,
        })
    return in_maps


def kernel(x, output_weights, output_bias):
    global _CACHED_NC
    if _CACHED_NC is None:
        _CACHED_NC = build_nc()
    nc = _CACHED_NC
    in_maps = _prepare_inputs(np.asarray(x, np.float32),
                              output_weights, output_bias)
    res = run_bass_kernel_spmd(nc, in_maps, core_ids=list(range(N_CORES)))
    outs = []
    for core in range(N_CORES):
        yc = res.results[core]["y"].reshape(-1)[:PER_CORE]
        outs.append(yc)
    return np.concatenate(outs).astype(np.float64)
